# revision 18
# baseline (speedup 1.0000x reference)
"""Additive (Bahdanau) attention scores on 8 Trainium2 NeuronCores.

Reference computation (per batch b, head h):
    qp = Q[b,h] @ Wq.T          # [Lq, Dh]
    kp = K[b,h] @ Wk.T          # [Lk, Dh]
    scores[q,k] = sum_e V[e] * tanh(qp[q,e] + kp[k,e] + bias[e])

Shapes: B=2, H=8, Lq=Lk=512, Dh=64. Data-parallel over the 16 (b,h)
heads -> 2 heads per core; W/V params replicated.

Device strategy per core (ACT-tanh bound, ~33.5M tanh/core):
  - e (64) is packed twice into the 128 SBUF partitions, so one
    partition-column carries the features of TWO q rows (a "pair").
  - kpt2[64h+e, k] = kp[k, e] duplicated into both halves.
  - qpb[64h+e, j]  = qp[2j+h, e] + bias[e]  (pair j's per-partition scalar).
  - DVE tensor_scalar_add broadcasts qpb[:, j] over kpt2 -> X (fp32, 2x mode).
  - One ACT Tanh instruction covers J=16 pairs (free dim 8192) -> bf16.
  - PE reduces over e with a sliding block-diagonal V matrix: G[128, 254]
    holds V at columns 126/127 (half h at col 126+h); lhsT = G[:, 126-2j :
    254-2j] has V at local columns (2j, 2j+1), so 64 accumulating matmuls
    build a [128, 512] PSUM score block (q on partitions), DMA'd to DRAM.
"""

import os
import time

import numpy as np
import ml_dtypes

import concourse.bass as bass
import concourse.tile as tile
from concourse import bacc, mybir
from concourse.bass_utils import run_bass_kernel_spmd

B, H, LQ, LK, DH = 2, 8, 512, 512, 64
N_CORES = 8
BH_PER_CORE = (B * H) // N_CORES  # 2
J = 32          # max q-pairs per ACT batch (free dim 512*J)
PAIRS_PER_BLK = 64   # q-pairs per [128, 512] PSUM score block
N_BLK = LQ // (2 * PAIRS_PER_BLK)  # 4 score blocks per head

F32 = mybir.dt.float32
BF16 = mybir.dt.bfloat16

# Module-level result of the last traced run (set when BASS_KERNEL_TRACE=1).
LAST_EXEC_TIME_NS = None
LAST_TRACE = None

_COMPILED_NC = None


def _build_nc():
    nc = bacc.Bacc("TRN2", target_bir_lowering=False, debug=False)

    qt = nc.dram_tensor("qt", [BH_PER_CORE, DH, LQ], BF16, kind="ExternalInput")
    kt = nc.dram_tensor("kt", [BH_PER_CORE, DH, LK], BF16, kind="ExternalInput")
    wqt = nc.dram_tensor("wqt", [DH, DH], BF16, kind="ExternalInput")
    wkt = nc.dram_tensor("wkt", [DH, DH], BF16, kind="ExternalInput")
    bias2 = nc.dram_tensor("bias2", [2 * DH, 1], F32, kind="ExternalInput")
    vg = nc.dram_tensor("vg", [2 * DH, 2], BF16, kind="ExternalInput")
    out = nc.dram_tensor("out", [BH_PER_CORE, LQ, LK], F32, kind="ExternalOutput")

    with tile.TileContext(nc) as tc:
        with (
            tc.tile_pool(name="const", bufs=1) as const_pool,
            tc.tile_pool(name="inp", bufs=2) as inp_pool,
            tc.tile_pool(name="prep", bufs=2) as prep_pool,
            tc.tile_pool(name="x", bufs=3) as x_pool,
            tc.tile_pool(name="t", bufs=2) as t_pool,
            tc.tile_pool(name="o", bufs=2) as o_pool,
            tc.tile_pool(name="proj_ps", bufs=2, space="PSUM") as proj_psum,
            tc.tile_pool(name="score_ps", bufs=2, space="PSUM") as score_psum,
        ):
            # Warm-up: preload the tanh ACT table set while input DMAs run.
            warm = const_pool.tile([128, 1], F32)
            nc.vector.memset(warm, 0.0)
            nc.scalar.activation(warm, warm, mybir.ActivationFunctionType.Tanh)

            # Head-0 Q/K first (critical path), split across two DMA queues.
            qts_all = []
            kts_all = []
            qts = inp_pool.tile([DH, LQ], BF16, tag="qts")
            nc.sync.dma_start(out=qts, in_=qt[0, :, :])
            qts_all.append(qts)
            kts = inp_pool.tile([DH, LK], BF16, tag="kts")
            nc.gpsimd.dma_start(out=kts, in_=kt[0, :, :])
            kts_all.append(kts)
            wq_sb = const_pool.tile([DH, DH], BF16)
            nc.sync.dma_start(out=wq_sb, in_=wqt[:, :])
            wk_sb = const_pool.tile([DH, DH], BF16)
            nc.gpsimd.dma_start(out=wk_sb, in_=wkt[:, :])
            for i in range(1, BH_PER_CORE):
                qts = inp_pool.tile([DH, LQ], BF16, tag="qts")
                nc.sync.dma_start(out=qts, in_=qt[i, :, :])
                qts_all.append(qts)
                kts = inp_pool.tile([DH, LK], BF16, tag="kts")
                nc.gpsimd.dma_start(out=kts, in_=kt[i, :, :])
                kts_all.append(kts)

            bias_sb = const_pool.tile([2 * DH, 1], F32)
            nc.gpsimd.dma_start(out=bias_sb, in_=bias2[:, :])
            # Sliding block-diagonal V band: G[p, 126 + p//64] = V[p % 64].
            g_sb = const_pool.tile([2 * DH, 254], BF16)
            nc.vector.memset(g_sb, 0.0)
            nc.gpsimd.dma_start(out=g_sb[:, 126:128], in_=vg[:, :])

            for i in range(BH_PER_CORE):
                qts = qts_all[i]
                kts = kts_all[i]

                # qpT[e, q] = sum_d Wq[e, d] * Q[q, d]; same for k.
                qp_ps = proj_psum.tile([DH, LQ], F32)
                nc.tensor.matmul(qp_ps, lhsT=wq_sb, rhs=qts, start=True, stop=True)
                kp_ps = proj_psum.tile([DH, LK], F32)
                nc.tensor.matmul(kp_ps, lhsT=wk_sb, rhs=kts, start=True, stop=True)

                # Pack: qpb[64h+e, j] = qpT[e, 2j+h] + bias[e]
                qpb = prep_pool.tile([2 * DH, LQ // 2], F32)
                qp_pairs = qp_ps.rearrange("e (j two) -> e two j", two=2)
                for h in range(2):
                    nc.vector.tensor_scalar_add(
                        out=qpb[64 * h : 64 * h + 64, :],
                        in0=qp_pairs[:, h, :],
                        scalar1=bias_sb[64 * h : 64 * h + 64, :],
                    )
                # kpt2[64h+e, k] = kpT[e, k]
                kpt2 = prep_pool.tile([2 * DH, LK], BF16)
                for h in range(2):
                    nc.vector.tensor_copy(
                        out=kpt2[64 * h : 64 * h + 64, :], in_=kp_ps
                    )

                for blk in range(N_BLK):
                    # Ramp in/out with small ACT batches so the pipeline
                    # fills fast and drains fast; J=32 in steady state.
                    if i == 0 and blk == 0:
                        batches = [8, 8, 16, 32]
                    elif i == BH_PER_CORE - 1 and blk == N_BLK - 1:
                        batches = [32, 16, 8, 4, 4]
                    else:
                        batches = [32, 32]
                    ps = score_psum.tile([128, LK], F32)
                    pr = 0
                    for bat_j in batches:
                        x_t = x_pool.tile([128, LK * J], BF16, tag="x_t")
                        for j in range(bat_j):
                            col = blk * PAIRS_PER_BLK + pr + j
                            nc.vector.tensor_scalar_add(
                                out=x_t[:, j * LK : (j + 1) * LK],
                                in0=kpt2,
                                scalar1=qpb[:, col : col + 1],
                            )
                        t_t = t_pool.tile([128, LK * J], BF16, tag="t_t")
                        nc.scalar.activation(
                            t_t[:, : bat_j * LK],
                            x_t[:, : bat_j * LK],
                            mybir.ActivationFunctionType.Tanh,
                        )
                        for j in range(bat_j):
                            p = pr + j
                            nc.tensor.matmul(
                                ps,
                                lhsT=g_sb[:, 126 - 2 * p : 254 - 2 * p],
                                rhs=t_t[:, j * LK : (j + 1) * LK],
                                start=(p == 0),
                                stop=(p == PAIRS_PER_BLK - 1),
                            )
                        pr += bat_j
                    o_t = o_pool.tile([128, LK], F32)
                    nc.vector.tensor_copy(out=o_t, in_=ps)
                    nc.sync.dma_start(
                        out=out[i, blk * 128 : (blk + 1) * 128, :], in_=o_t
                    )

    nc.compile()
    return nc


def kernel(Q, K, W_weight, W_bias, V_weight):
    global LAST_EXEC_TIME_NS, LAST_TRACE, _COMPILED_NC

    Q = np.asarray(Q, dtype=np.float32)
    K = np.asarray(K, dtype=np.float32)
    W_weight = np.asarray(W_weight, dtype=np.float32)
    W_bias = np.asarray(W_bias, dtype=np.float32)
    V_weight = np.asarray(V_weight, dtype=np.float32)

    # Host-side shard prep (layout only; all FLOPs run on device).
    qt_all = np.ascontiguousarray(
        Q.reshape(B * H, LQ, DH).transpose(0, 2, 1).astype(ml_dtypes.bfloat16)
    )  # [16, 64, 512]
    kt_all = np.ascontiguousarray(
        K.reshape(B * H, LK, DH).transpose(0, 2, 1).astype(ml_dtypes.bfloat16)
    )
    wqt = np.ascontiguousarray(
        W_weight[:, :DH].T.astype(ml_dtypes.bfloat16)
    )  # [d, e] = Wq[e, d]
    wkt = np.ascontiguousarray(W_weight[:, DH:].T.astype(ml_dtypes.bfloat16))
    bias2 = np.tile(W_bias, 2)[:, None].astype(np.float32)  # [128, 1]
    vg = np.zeros((2 * DH, 2), dtype=ml_dtypes.bfloat16)
    vg[:DH, 0] = V_weight.astype(ml_dtypes.bfloat16)
    vg[DH:, 1] = V_weight.astype(ml_dtypes.bfloat16)

    if _COMPILED_NC is None:
        _COMPILED_NC = _build_nc()
    nc = _COMPILED_NC

    in_maps = []
    for c in range(N_CORES):
        sl = slice(c * BH_PER_CORE, (c + 1) * BH_PER_CORE)
        in_maps.append(
            {
                "qt": np.ascontiguousarray(qt_all[sl]),
                "kt": np.ascontiguousarray(kt_all[sl]),
                "wqt": wqt,
                "wkt": wkt,
                "bias2": bias2,
                "vg": vg,
            }
        )

    trace = bool(int(os.environ.get("BASS_KERNEL_TRACE", "0")))
    res = None
    last_exc = None
    for attempt in range(3):
        try:
            res = run_bass_kernel_spmd(
                nc, in_maps, core_ids=list(range(N_CORES)), trace=trace
            )
            break
        except Exception as e:  # transient NRT/device errors on fresh NEFFs
            last_exc = e
            time.sleep(2.0)
    if res is None:
        raise last_exc
    LAST_EXEC_TIME_NS = res.exec_time_ns
    LAST_TRACE = res

    full = np.concatenate(
        [res.results[c]["out"] for c in range(N_CORES)], axis=0
    )  # [16, 512, 512]
    return full.reshape(B, H, LQ, LK)


# revision 25
# speedup vs baseline: 1.0021x; 1.0021x over previous
"""Additive (Bahdanau) attention scores on 8 Trainium2 NeuronCores.

Reference computation (per batch b, head h):
    qp = Q[b,h] @ Wq.T          # [Lq, Dh]
    kp = K[b,h] @ Wk.T          # [Lk, Dh]
    scores[q,k] = sum_e V[e] * tanh(qp[q,e] + kp[k,e] + bias[e])

Shapes: B=2, H=8, Lq=Lk=512, Dh=64. Data-parallel over the 16 (b,h)
heads -> 2 heads per core; W/V params replicated.

Device strategy per core (ACT-tanh bound, ~33.5M tanh/core):
  - e (64) is packed twice into the 128 SBUF partitions, so one
    partition-column carries the features of TWO q rows (a "pair").
  - kpt2[64h+e, k] = kp[k, e] duplicated into both halves.
  - qpb[64h+e, j]  = qp[2j+h, e] + bias[e]  (pair j's per-partition scalar).
  - DVE tensor_scalar_add broadcasts qpb[:, j] over kpt2 -> X (fp32, 2x mode).
  - One ACT Tanh instruction covers J=16 pairs (free dim 8192) -> bf16.
  - PE reduces over e with a sliding block-diagonal V matrix: G[128, 254]
    holds V at columns 126/127 (half h at col 126+h); lhsT = G[:, 126-2j :
    254-2j] has V at local columns (2j, 2j+1), so 64 accumulating matmuls
    build a [128, 512] PSUM score block (q on partitions), DMA'd to DRAM.
"""

import os
import time

import numpy as np
import ml_dtypes

import concourse.bass as bass
import concourse.tile as tile
from concourse import bacc, mybir
from concourse.bass_utils import run_bass_kernel_spmd

B, H, LQ, LK, DH = 2, 8, 512, 512, 64
N_CORES = 8
BH_PER_CORE = (B * H) // N_CORES  # 2
J = 32          # max q-pairs per ACT batch (free dim 512*J)
PAIRS_PER_BLK = 64   # q-pairs per [128, 512] PSUM score block
N_BLK = LQ // (2 * PAIRS_PER_BLK)  # 4 score blocks per head

F32 = mybir.dt.float32
BF16 = mybir.dt.bfloat16

# Module-level result of the last traced run (set when BASS_KERNEL_TRACE=1).
LAST_EXEC_TIME_NS = None
LAST_TRACE = None

_COMPILED_NC = None


def _build_nc():
    nc = bacc.Bacc("TRN2", target_bir_lowering=False, debug=False)

    qt = nc.dram_tensor("qt", [BH_PER_CORE, DH, LQ], BF16, kind="ExternalInput")
    kt = nc.dram_tensor("kt", [BH_PER_CORE, DH, LK], BF16, kind="ExternalInput")
    wqt = nc.dram_tensor("wqt", [DH, DH], BF16, kind="ExternalInput")
    # Wk^T duplicated along columns: wk2[d, 64h+e] = Wk[e, d], so the k-side
    # projection lands in PSUM already duplicated into both partition halves.
    wk2 = nc.dram_tensor("wk2", [DH, 2 * DH], BF16, kind="ExternalInput")
    bias2 = nc.dram_tensor("bias2", [2 * DH, 1], F32, kind="ExternalInput")
    vg = nc.dram_tensor("vg", [2 * DH, 2], BF16, kind="ExternalInput")
    out = nc.dram_tensor("out", [BH_PER_CORE, LQ, LK], F32, kind="ExternalOutput")

    with tile.TileContext(nc) as tc:
        with (
            tc.tile_pool(name="const", bufs=1) as const_pool,
            tc.tile_pool(name="inp", bufs=2) as inp_pool,
            tc.tile_pool(name="prep", bufs=2) as prep_pool,
            tc.tile_pool(name="x", bufs=3) as x_pool,
            tc.tile_pool(name="t", bufs=2) as t_pool,
            tc.tile_pool(name="o", bufs=2) as o_pool,
            tc.tile_pool(name="proj_ps", bufs=2, space="PSUM") as proj_psum,
            tc.tile_pool(name="score_ps", bufs=2, space="PSUM") as score_psum,
        ):
            # Warm-up: preload the tanh ACT table set while input DMAs run.
            warm = const_pool.tile([128, 1], F32)
            nc.vector.memset(warm, 0.0)
            nc.scalar.activation(warm, warm, mybir.ActivationFunctionType.Tanh)

            # Head-0 Q/K first (critical path), split across two DMA queues.
            qts_all = []
            kts_all = []
            qts = inp_pool.tile([DH, LQ], BF16, tag="qts")
            nc.sync.dma_start(out=qts, in_=qt[0, :, :])
            qts_all.append(qts)
            kts = inp_pool.tile([DH, LK], BF16, tag="kts")
            nc.gpsimd.dma_start(out=kts, in_=kt[0, :, :])
            kts_all.append(kts)
            wq_sb = const_pool.tile([DH, DH], BF16)
            nc.sync.dma_start(out=wq_sb, in_=wqt[:, :])
            wk_sb = const_pool.tile([DH, 2 * DH], BF16)
            nc.gpsimd.dma_start(out=wk_sb, in_=wk2[:, :])
            for i in range(1, BH_PER_CORE):
                qts = inp_pool.tile([DH, LQ], BF16, tag="qts")
                nc.sync.dma_start(out=qts, in_=qt[i, :, :])
                qts_all.append(qts)
                kts = inp_pool.tile([DH, LK], BF16, tag="kts")
                nc.gpsimd.dma_start(out=kts, in_=kt[i, :, :])
                kts_all.append(kts)

            bias_sb = const_pool.tile([2 * DH, 1], F32)
            nc.gpsimd.dma_start(out=bias_sb, in_=bias2[:, :])
            # Sliding block-diagonal V band: G[p, 126 + p//64] = V[p % 64].
            g_sb = const_pool.tile([2 * DH, 254], BF16)
            nc.vector.memset(g_sb, 0.0)
            nc.gpsimd.dma_start(out=g_sb[:, 126:128], in_=vg[:, :])

            blk_plans = []
            for i in range(BH_PER_CORE):
                qts = qts_all[i]
                kts = kts_all[i]

                # kp2[64h+e, k] = sum_d Wk[e, d] * K[k, d] (both halves at once)
                kp_ps = proj_psum.tile([2 * DH, LK], F32)
                nc.tensor.matmul(kp_ps, lhsT=wk_sb, rhs=kts, start=True, stop=True)
                # qpT[e, q] = sum_d Wq[e, d] * Q[q, d]
                qp_ps = proj_psum.tile([DH, LQ], F32)
                nc.tensor.matmul(qp_ps, lhsT=wq_sb, rhs=qts, start=True, stop=True)

                # kpt2[64h+e, k] = kpT[e, k]
                kpt2 = prep_pool.tile([2 * DH, LK], BF16)
                nc.vector.tensor_copy(out=kpt2, in_=kp_ps)
                # Pack: qpb[64h+e, j] = qpT[e, 2j+h] + bias[e]
                qpb = prep_pool.tile([2 * DH, LQ // 2], F32)
                qp_pairs = qp_ps.rearrange("e (j two) -> e two j", two=2)
                for h in range(2):
                    nc.vector.tensor_scalar_add(
                        out=qpb[64 * h : 64 * h + 64, :],
                        in0=qp_pairs[:, h, :],
                        scalar1=bias_sb[64 * h : 64 * h + 64, :],
                    )

                blk_plans.append((i, kpt2, qpb))

            def flush_pending(pending):
                ps, i, blk = pending
                o_t = o_pool.tile([128, LK], F32, tag="o_t")
                nc.vector.tensor_copy(out=o_t, in_=ps)
                nc.sync.dma_start(
                    out=out[i, blk * 128 : (blk + 1) * 128, :], in_=o_t
                )

            n_total = BH_PER_CORE * N_BLK
            pending = None
            for flat in range(n_total):
                i, blk = flat // N_BLK, flat % N_BLK
                _, kpt2, qpb = blk_plans[i]
                # Ramp in/out with small ACT batches so the pipeline
                # fills fast and drains fast; J=32 in steady state.
                if flat == 0:
                    batches = [4, 8, 8, 12, 32]
                elif flat == n_total - 1:
                    batches = [32, 16, 8, 4, 4]
                else:
                    batches = [32, 32]
                ps = score_psum.tile([128, LK], F32, tag="ps")
                pr = 0
                for bat_j in batches:
                    x_t = x_pool.tile([128, LK * J], BF16, tag="x_t")
                    for j in range(bat_j):
                        col = blk * PAIRS_PER_BLK + pr + j
                        nc.vector.tensor_scalar_add(
                            out=x_t[:, j * LK : (j + 1) * LK],
                            in0=kpt2,
                            scalar1=qpb[:, col : col + 1],
                        )
                    t_t = t_pool.tile([128, LK * J], BF16, tag="t_t")
                    nc.scalar.activation(
                        t_t[:, : bat_j * LK],
                        x_t[:, : bat_j * LK],
                        mybir.ActivationFunctionType.Tanh,
                    )
                    for j in range(bat_j):
                        p = pr + j
                        nc.tensor.matmul(
                            ps,
                            lhsT=g_sb[:, 126 - 2 * p : 254 - 2 * p],
                            rhs=t_t[:, j * LK : (j + 1) * LK],
                            start=(p == 0),
                            stop=(p == PAIRS_PER_BLK - 1),
                        )
                    pr += bat_j
                # Defer the PSUM->SBUF readout of the previous block until
                # after this block's adds so it can't head-of-line-block the
                # DVE queue while waiting on the previous block's matmuls.
                if pending is not None:
                    flush_pending(pending)
                pending = (ps, i, blk)
            flush_pending(pending)

    nc.compile()
    return nc


def kernel(Q, K, W_weight, W_bias, V_weight):
    global LAST_EXEC_TIME_NS, LAST_TRACE, _COMPILED_NC

    Q = np.asarray(Q, dtype=np.float32)
    K = np.asarray(K, dtype=np.float32)
    W_weight = np.asarray(W_weight, dtype=np.float32)
    W_bias = np.asarray(W_bias, dtype=np.float32)
    V_weight = np.asarray(V_weight, dtype=np.float32)

    # Host-side shard prep (layout only; all FLOPs run on device).
    qt_all = np.ascontiguousarray(
        Q.reshape(B * H, LQ, DH).transpose(0, 2, 1).astype(ml_dtypes.bfloat16)
    )  # [16, 64, 512]
    kt_all = np.ascontiguousarray(
        K.reshape(B * H, LK, DH).transpose(0, 2, 1).astype(ml_dtypes.bfloat16)
    )
    wqt = np.ascontiguousarray(
        W_weight[:, :DH].T.astype(ml_dtypes.bfloat16)
    )  # [d, e] = Wq[e, d]
    wkt = W_weight[:, DH:].T.astype(ml_dtypes.bfloat16)  # [d, e]
    wk2 = np.ascontiguousarray(np.concatenate([wkt, wkt], axis=1))  # [d, 128]
    bias2 = np.tile(W_bias, 2)[:, None].astype(np.float32)  # [128, 1]
    vg = np.zeros((2 * DH, 2), dtype=ml_dtypes.bfloat16)
    vg[:DH, 0] = V_weight.astype(ml_dtypes.bfloat16)
    vg[DH:, 1] = V_weight.astype(ml_dtypes.bfloat16)

    if _COMPILED_NC is None:
        _COMPILED_NC = _build_nc()
    nc = _COMPILED_NC

    in_maps = []
    for c in range(N_CORES):
        sl = slice(c * BH_PER_CORE, (c + 1) * BH_PER_CORE)
        in_maps.append(
            {
                "qt": np.ascontiguousarray(qt_all[sl]),
                "kt": np.ascontiguousarray(kt_all[sl]),
                "wqt": wqt,
                "wk2": wk2,
                "bias2": bias2,
                "vg": vg,
            }
        )

    trace = bool(int(os.environ.get("BASS_KERNEL_TRACE", "0")))
    res = None
    last_exc = None
    for attempt in range(3):
        try:
            res = run_bass_kernel_spmd(
                nc, in_maps, core_ids=list(range(N_CORES)), trace=trace
            )
            break
        except Exception as e:  # transient NRT/device errors on fresh NEFFs
            last_exc = e
            time.sleep(2.0)
    if res is None:
        raise last_exc
    LAST_EXEC_TIME_NS = res.exec_time_ns
    LAST_TRACE = res

    full = np.concatenate(
        [res.results[c]["out"] for c in range(N_CORES)], axis=0
    )  # [16, 512, 512]
    return full.reshape(B, H, LQ, LK)
